# revision 1
# baseline (speedup 1.0000x reference)
"""AnyVariateAttention Trainium2 kernel (8 NeuronCores, SPMD).

Sharding: 16 (batch, head) pairs / 8 cores -> each core computes 2 adjacent
heads of one batch (core c: batch c//4, heads 2*(c%4), 2*(c%4)+1).
Per core: QKV projection (transposed layouts), partial RoPE, flash-style
attention with transposed scores (S^T tiles [k,q]), block-bias folded into the
exp via the ACT bias port, softmax denominator via a ones-column appended to V,
normalization via reciprocal + ones-rank-1 broadcast matmul, and a partial
output projection. Host sums the 4 per-batch partials and transposes.
"""

import sys
import os
import numpy as np

for _p in ("/opt/trn_rl_repo",):
    if _p not in sys.path:
        sys.path.insert(0, _p)

import ml_dtypes

BF16 = ml_dtypes.bfloat16

B, N, D, H, HD = 2, 4096, 256, 8, 32
SEQ = 512
SCALE = HD ** -0.5
NCORES = 8
SCHRAUD_A = 184.6650390625  # 128 * log2(e)
SCHRAUD_B = 16256.0 - 7.4
# 116 of 256 exp tiles on DVE via bit-trick exp (well-mixed hash)

_NC_CACHE = {}


def _build_nc(stage=4):
    import concourse.bass as bass
    import concourse.tile as tile
    from concourse import bacc, mybir
    from concourse.bass import ts

    from concourse.alu_op_type import AluOpType
    bf = mybir.dt.bfloat16
    f32 = mybir.dt.float32
    i16 = mybir.dt.int16
    EXP = mybir.ActivationFunctionType.Exp

    nc = bacc.Bacc("TRN2", target_bir_lowering=False, debug=False, num_devices=NCORES)

    hsT_d = nc.declare_dram_parameter("hsT", [D, N], bf, isOutput=False)
    wq_d = nc.declare_dram_parameter("wq", [D, 64], bf, isOutput=False)
    wk_d = nc.declare_dram_parameter("wk", [D, 64], bf, isOutput=False)
    wv_d = nc.declare_dram_parameter("wv", [D, 64], bf, isOutput=False)
    wo_d = nc.declare_dram_parameter("wo", [32, 2 * D], bf, isOutput=False)
    cos_d = nc.declare_dram_parameter("rope_cos", [128, N], bf, isOutput=False)
    sin_d = nc.declare_dram_parameter("rope_sin", [128, N], bf, isOutput=False)
    bias_d = nc.declare_dram_parameter("biases", [128, 4], f32, isOutput=False)
    out_d = nc.declare_dram_parameter("outT", [D, N], f32, isOutput=True)

    NT = N // 512  # 8 q-tiles of 512
    NCP = N // 256  # 16 chunk-pairs (2x128 k rows each)

    with tile.TileContext(nc) as tc:
        from contextlib import ExitStack

        with ExitStack() as ctx:
            const = ctx.enter_context(tc.tile_pool(name="const", bufs=1))

            hs_sb = const.tile([128, 2, N], bf, tag="hs_sb")
            wq_sb = const.tile([128, 2, 64], bf, tag="wq_sb")
            wk_sb = const.tile([128, 2, 64], bf, tag="wk_sb")
            wv_sb = const.tile([128, 2, 64], bf, tag="wv_sb")
            wo_sb = const.tile([32, 2, D], bf, tag="wo_sb")
            cos_sb = const.tile([128, N], bf, tag="cos_sb")
            sin_sb = const.tile([128, N], bf, tag="sin_sb")
            bias_sb = const.tile([128, 4], f32, tag="bias_sb")
            schraud_sb = const.tile([128, 4], f32, tag="schraud_sb")
            ones_sb = const.tile([1, 32], f32, tag="ones_sb")
            qk_sb = const.tile([128, N], bf, tag="qk_sb")
            tmp_sb = const.tile([128, N], bf, tag="tmp_sb")
            Qd = const.tile([128, N], bf, tag="Qd")
            Kd = const.tile([128, N], bf, tag="Kd")
            # v tiles: [k-chunk 128, chunk, head, 35] with [v(32) | ones(1) | pad(2)]
            v_sb = const.tile([128, 32, 2, 35], bf, tag="v_sb")

            # --- input DMAs ---
            for d in range(2):
                nc.sync.dma_start(wq_sb[:, d, :], wq_d[ts(d, 128), :])
                nc.sync.dma_start(hs_sb[:, d, 0:1024],
                                  hsT_d[ts(d, 128), 0:1024])
                nc.sync.dma_start(wk_sb[:, d, :], wk_d[ts(d, 128), :])
                for cb in range(1, 4):
                    nc.sync.dma_start(hs_sb[:, d, ts(cb, 1024)],
                                      hsT_d[ts(d, 128), ts(cb, 1024)])
                nc.sync.dma_start(wv_sb[:, d, :], wv_d[ts(d, 128), :])
            nc.sync.dma_start(
                wo_sb[:], wo_d[:].rearrange("p (h j) -> p h j", h=2))
            nc.sync.dma_start(bias_sb[:], bias_d[:])
            for cb in range(4):
                nc.sync.dma_start(cos_sb[:, ts(cb, 1024)],
                                  cos_d[:, ts(cb, 1024)])
                nc.sync.dma_start(sin_sb[:, ts(cb, 1024)],
                                  sin_d[:, ts(cb, 1024)])
            nc.vector.tensor_scalar(
                schraud_sb[:], bias_sb[:], SCHRAUD_A, SCHRAUD_B,
                AluOpType.mult, AluOpType.add)
            nc.vector.memset(ones_sb[:], 1.0)
            nc.vector.memset(v_sb[:, :, :, 32:33], 1.0)

            # --- phase 1: q/k projections (transposed layout) + v (natural) ---
            with tc.tile_pool(name="qkp", bufs=2, space="PSUM") as qkp:
                for t in range(NT):
                    ps = qkp.tile([128, 512], f32, tag="qkps")
                    for d in range(2):
                        nc.tensor.matmul(
                            ps[0:64, :], lhsT=wq_sb[:, d, :],
                            rhs=hs_sb[:, d, ts(t, 512)],
                            start=(d == 0), stop=(d == 1), tile_position=(0, 0))
                        nc.tensor.matmul(
                            ps[64:128, :], lhsT=wk_sb[:, d, :],
                            rhs=hs_sb[:, d, ts(t, 512)],
                            start=(d == 0), stop=(d == 1), tile_position=(0, 64))
                    nc.scalar.copy(qk_sb[:, ts(t, 512)], ps[:])
            if stage == 1:
                ob = out_d[:].bitcast(bf)
                nc.sync.dma_start(ob[0:128, 0:N], qk_sb[:])
                nc.sync.dma_start(
                    ob[128 : 128 + 128, 0 : 32 * 2 * 35],
                    v_sb[:].rearrange("p a b c -> p (a b c)"))

            # --- phase 2: partial RoPE on q and k (rows: qA qB kA kB) ---
            if stage >= 2:
                for cb in range(4):
                    cs = ts(cb, 1024)
                    for g in (0, 2, 1, 3):
                        b0 = 32 * g
                        nc.sync.dma_start(
                            tmp_sb[b0:b0 + 16, cs], qk_sb[b0 + 16:b0 + 32, cs])
                        nc.sync.dma_start(
                            tmp_sb[b0 + 16:b0 + 32, cs], qk_sb[b0:b0 + 16, cs])
                    nc.vector.tensor_mul(tmp_sb[:, cs], tmp_sb[:, cs], sin_sb[:, cs])
                    nc.vector.tensor_mul(qk_sb[:, cs], qk_sb[:, cs], cos_sb[:, cs])
                    nc.vector.tensor_add(qk_sb[:, cs], qk_sb[:, cs], tmp_sb[:, cs])
                    # duplicate q and k to both halves (4-way row tiling)
                    nc.sync.dma_start(Qd[0:64, cs], qk_sb[0:64, cs])
                    nc.sync.dma_start(Qd[64:128, cs], qk_sb[0:64, cs])
                    nc.sync.dma_start(Kd[0:64, cs], qk_sb[64:128, cs])
                    nc.sync.dma_start(Kd[64:128, cs], qk_sb[64:128, cs])

            # v projection after rope emission so DVE ropes immediately
            with tc.tile_pool(name="vpp", bufs=2, space="PSUM") as vpp:
                for tv in range(32):
                    vp = vpp.tile([128, 64], f32, tag="vps")
                    for d in range(2):
                        nc.tensor.matmul(
                            vp[:], lhsT=hs_sb[:, d, ts(tv, 128)],
                            rhs=wv_sb[:, d, :],
                            start=(d == 0), stop=(d == 1))
                    nc.scalar.copy(
                        v_sb[:, tv, :, 0:32],
                        vp[:].rearrange("p (h x) -> p h x", h=2))

            if stage == 2:
                ob = out_d[:].bitcast(bf)
                nc.sync.dma_start(ob[0:128, 0:N], Qd[:])
                nc.sync.dma_start(ob[128:256, 0:N], Kd[:])

            # --- phase 3: attention main loop (software-pipelined
            # emission: PV trails scores by one step, the PE parts of the
            # norm/oproj chain trail by another, so the in-order PE queue
            # never sits behind an exp or reciprocal dependency) ---
            n_t = 0 if stage < 3 else (1 if stage == 3 else NT)
            with tc.tile_pool(name="spp", bufs=3, space="PSUM") as spp, \
                 tc.tile_pool(name="pvp", bufs=2, space="PSUM") as pvp, \
                 tc.tile_pool(name="ptp", bufs=7) as ptp, \
                 tc.tile_pool(name="mgp", bufs=3) as mgp, \
                 tc.tile_pool(name="rcp", bufs=4) as rcp, \
                 tc.tile_pool(name="ntp", bufs=3) as ntp:
                pv_tiles = {}

                def emit_scores_exp(t, cp):
                    sp = [spp.tile([128, 1024], f32, tag="sp",
                                   name=f"sp{t}_{cp}_{h}") for h in range(2)]
                    for g in (0, 2, 1, 3):
                        c = 2 * cp + (g // 2)
                        h = g % 2
                        half = g // 2
                        nc.tensor.matmul(
                            sp[h][:, ts(half, 512)],
                            lhsT=Kd[ts(g, 32), ts(c, 128)],
                            rhs=Qd[ts(g, 32), ts(t, 512)],
                            start=True, stop=True,
                            tile_position=(32 * g, 0))
                    pts = [None, None]
                    same = (cp // 2) == t
                    for h in range(2):
                        col = 2 * h + (0 if same else 1)
                        idx = (t * NCP + cp) * 2 + h
                        on_dve = ((idx * 21) % 50 < 23) and idx not in (62, 191)
                        if on_dve:
                            pt = ptp.tile([128, 1024], i16, tag="pt",
                                          name=f"pti{t}_{cp}_{h}")
                            nc.vector.tensor_scalar(
                                pt[:], sp[h][:], SCHRAUD_A,
                                schraud_sb[:, col:col + 1],
                                AluOpType.mult, AluOpType.add)
                            pts[h] = pt[:].bitcast(bf)
                        else:
                            pt = ptp.tile([128, 1024], bf, tag="pt",
                                          name=f"pt{t}_{cp}_{h}")
                            nc.scalar.activation(
                                pt[:], sp[h][:], EXP,
                                bias=bias_sb[:, col:col + 1], scale=1.0)
                            pts[h] = pt[:]
                    return pts

                def emit_pv(t, cp, pts):
                    if cp == 0:
                        pv_tiles[t] = [
                            pvp.tile([128, 512], f32, tag="pv",
                                     name=f"pv{t}_{h}") for h in range(2)]
                    pv = pv_tiles[t]
                    for h in range(2):
                        for j in range(2):
                            c = 2 * cp + j
                            pbase = 0 if j == 0 else 64
                            nc.tensor.matmul(
                                pv[h][pbase:pbase + 33, :],
                                lhsT=v_sb[:, c, h, 0:33],
                                rhs=pts[h][:, ts(j, 512)],
                                start=(cp == 0), stop=(cp == NCP - 1),
                                tile_position=(0, pbase))

                def emit_front(t):
                    pv = pv_tiles[t]
                    st = []
                    for h in range(2):
                        stage0 = mgp.tile([33, 512], f32, tag="stage0",
                                          name=f"stage0_{t}_{h}")
                        nc.scalar.copy(stage0[:], pv[h][64:97, :])
                        merged = mgp.tile([33, 512], f32, tag="mg",
                                          name=f"mg{t}_{h}")
                        nc.vector.tensor_add(
                            merged[:], pv[h][0:33, :], stage0[:])
                        den0 = rcp.tile([1, 512], f32, tag="den0",
                                        name=f"den0_{t}_{h}")
                        nc.sync.dma_start(den0[:], merged[32:33, :])
                        recip = rcp.tile([1, 512], f32, tag="rc",
                                         name=f"rc{t}_{h}")
                        nc.vector.reciprocal_approx_fast(
                            out=recip[:], in_=den0[:])
                        st.append((merged, recip))
                    return (t, st)

                def emit_tail(tail):
                    t, st = tail
                    numts = []
                    for h in range(2):
                        merged, recip = st[h]
                        bc = pvp.tile([32, 512], f32, tag="pv",
                                      name=f"bc{t}_{h}")
                        nc.tensor.matmul(
                            bc[:], lhsT=ones_sb[:], rhs=recip[:],
                            start=True, stop=True)
                        numt = ntp.tile([32, 512], bf, tag=f"numt{h}",
                                        name=f"numt{t}_{h}")
                        nc.vector.tensor_mul(numt[:], merged[0:32, :], bc[:])
                        numts.append(numt)
                    for jc in range(2):
                        op_ps = pvp.tile([128, 512], f32, tag="pv",
                                         name=f"op{t}_{jc}")
                        for h in range(2):
                            nc.tensor.matmul(
                                op_ps[:], lhsT=wo_sb[:, h, ts(jc, 128)],
                                rhs=numts[h][:],
                                start=(h == 0), stop=(h == 1))
                        op_sb = ntp.tile([128, 512], f32, tag="opsb",
                                         name=f"opsb{t}_{jc}")
                        nc.scalar.copy(op_sb[:], op_ps[:])
                        nc.sync.dma_start(
                            out_d[ts(jc, 128), ts(t, 512)], op_sb[:])

                from collections import deque
                pend_pv = deque()  # (t, cp, pts), emitted 2 steps later
                prev_front = None  # t
                prev_tail = None   # (t, st)
                for t in range(n_t):
                    for cp in range(NCP):
                        pts = emit_scores_exp(t, cp)
                        if prev_tail is not None:
                            emit_tail(prev_tail)
                            prev_tail = None
                        if len(pend_pv) >= 1:
                            pt_, pc_, pp_ = pend_pv.popleft()
                            emit_pv(pt_, pc_, pp_)
                            if pc_ == NCP - 1:
                                prev_front = pt_
                        if prev_front is not None:
                            prev_tail = emit_front(prev_front)
                            prev_front = None
                        pend_pv.append((t, cp, pts))
                while pend_pv:
                    pt_, pc_, pp_ = pend_pv.popleft()
                    emit_pv(pt_, pc_, pp_)
                    if prev_tail is not None:
                        emit_tail(prev_tail)
                        prev_tail = None
                    if pc_ == NCP - 1:
                        prev_front = pt_
                    if prev_front is not None:
                        prev_tail = emit_front(prev_front)
                        prev_front = None
                if prev_tail is not None:
                    emit_tail(prev_tail)
    nc.compile()
    return nc


def _rope_tables():
    j = np.arange(8, dtype=np.float64)
    inv = 10000.0 ** (-(2.0 * j / HD))  # [8]
    ang = np.arange(N, dtype=np.float64)[None, :] * inv[:, None]  # [8, N]
    cosb = np.ones((32, N), dtype=np.float64)
    sinb = np.zeros((32, N), dtype=np.float64)
    cosb[0:8] = np.cos(ang)
    cosb[16:24] = np.cos(ang)
    sinb[0:8] = -np.sin(ang)
    sinb[16:24] = np.sin(ang)
    cos128 = np.tile(cosb, (4, 1)).astype(BF16)
    sin128 = np.tile(sinb, (4, 1)).astype(BF16)
    return cos128, sin128


def kernel(**inputs):
    hs = np.asarray(inputs["hidden_states"], dtype=np.float32)
    qw = np.asarray(inputs["q_w"], dtype=np.float32)
    kw = np.asarray(inputs["k_w"], dtype=np.float32)
    vw = np.asarray(inputs["v_w"], dtype=np.float32)
    ow = np.asarray(inputs["o_w"], dtype=np.float32)
    ob = np.asarray(inputs["o_b"], dtype=np.float32)
    qb = np.asarray(inputs["q_b"], dtype=np.float32)
    kb = np.asarray(inputs["k_b"], dtype=np.float32)
    vb = np.asarray(inputs["v_b"], dtype=np.float32)
    ab = np.asarray(inputs["attention_biases"], dtype=np.float32)
    seq = int(np.asarray(inputs["sequence_length"]))
    assert seq == SEQ, f"kernel compiled for sequence_length={SEQ}, got {seq}"
    assert hs.shape == (B, N, D)
    assert not (np.any(qb) or np.any(kb) or np.any(vb)), "nonzero qkv bias unsupported"

    stage = int(os.environ.get("KERNEL_STAGE", "4"))
    if ("nc", stage) not in _NC_CACHE:
        _NC_CACHE[("nc", stage)] = _build_nc(stage)
    nc = _NC_CACHE[("nc", stage)]

    cos128, sin128 = _rope_tables()
    in_maps = []
    for c in range(NCORES):
        b = c // 4
        h0 = 2 * (c % 4)
        rows = slice(h0 * HD, h0 * HD + 2 * HD)
        in_maps.append({
            "hsT": np.ascontiguousarray(hs[b].T).astype(BF16),
            "wq": np.ascontiguousarray((qw[rows, :] * SCALE).T).astype(BF16),
            "wk": np.ascontiguousarray(kw[rows, :].T).astype(BF16),
            "wv": np.ascontiguousarray(vw[rows, :].T).astype(BF16),
            "wo": np.ascontiguousarray(
                ow[:, rows].T.reshape(2, 32, D).transpose(1, 0, 2)
                .reshape(32, 2 * D)).astype(BF16),
            "rope_cos": cos128,
            "rope_sin": sin128,
            "biases": np.ascontiguousarray(
                np.broadcast_to(ab[h0:h0 + 2].reshape(1, 4), (128, 4))
            ).astype(np.float32),
        })

    global _LAST_IN_MAPS, _LAST_RESULTS
    _LAST_IN_MAPS = in_maps
    from concourse.bass_utils import run_bass_kernel_spmd
    res = run_bass_kernel_spmd(nc, in_maps, core_ids=list(range(NCORES)))
    _LAST_RESULTS = res.results
    out = np.zeros((B, N, D), dtype=np.float32)
    for c in range(NCORES):
        out[c // 4] += res.results[c]["outT"].T.astype(np.float32)
    out += ob[None, None, :]
    return out



# revision 53
# speedup vs baseline: 1.2664x; 1.2664x over previous
"""AnyVariateAttention Trainium2 kernel (8 NeuronCores, SPMD).

Sharding: 16 (batch, head) pairs / 8 cores -> each core computes 2 adjacent
heads of one batch (core c: batch c//4, heads 2*(c%4), 2*(c%4)+1).

Per core: QKV projection (transposed layouts), partial RoPE, flash-style
attention with transposed scores (S^T tiles [k,q]), block-bias folded into
the exp (ACT bias port / Schraudolph bias), softmax denominator via a
ones-column appended to V.

v2 restructure vs baseline:
- PV uses P^T as the *stationary* operand ([128k x 128q]) and V|ones as the
  *moving* operand (33 wide): 33 cycles/chunk instead of 512 -> ~4x less PE
  time in PV.
- PV output lands as O [q, hd]; normalization is a per-partition
  reciprocal+scale fused into the PSUM->SBUF eviction, then O is transposed
  back to [hd, q] on the PE (identity-matmul transpose) for the out-proj.
- exp is spread across ACT (exact), DVE and GpSimd/Pool (Schraudolph
  bit-trick) with a static greedy schedule balancing per-engine load.
- PSUM: one 3-slot ring of 4KB slots (score tiles / transpose tile / oproj
  tiles) + a 2-slot pool of per-t PV accumulator banks (both heads x 4
  q-slices in one 2KB bank; single start=True marks the bank, the other
  accumulation groups use start=False + skip_group_check).
"""

import sys
import os
import numpy as np

for _p in ("/opt/trn_rl_repo",):
    if _p not in sys.path:
        sys.path.insert(0, _p)

import ml_dtypes

BF16 = ml_dtypes.bfloat16

B, N, D, H, HD = 2, 4096, 256, 8, 32
SEQ = 512
SCALE = HD ** -0.5
NCORES = 8
SCHRAUD_A = 184.6650390625  # 128 * log2(e)
SCHRAUD_B = 16256.0 - 7.4

_NC_CACHE = {}


def _exp_schedule():
    """Static (t, cp, h) -> engine schedule for the 256 exp tiles
    ([128, 1024] each). GPSIMD cannot read PSUM, so only ACT (exact exp)
    and DVE (Schraudolph) can drain the score banks.

    Greedy finish-time balancing with cost-model estimates (ns): per-tile
    busy ACT 1038 / DVE 1192; fixed loads ACT ~20us (qk/v/op evictions),
    DVE ~25us (recip, norm, numt evict, rope cb0)."""
    costs = {"A": 1038.0, "D": 1192.0}
    load = {"A": 20000.0, "D": 29500.0}
    sched = []
    for i in range(256):
        cp = (i // 2) % 16
        if cp == 15 and i % 2 == 1:
            # keep DVE free right when each q-tile's recip+norm chain lands
            e = "A"
        else:
            e = min("AD", key=lambda e: load[e] + costs[e])
        sched.append(e)
        load[e] += costs[e]
    return sched


def _build_nc(stage=4, pvd=3):
    import concourse.bass as bass
    import concourse.tile as tile
    from concourse import bacc, mybir
    from concourse.bass import ts

    from concourse.alu_op_type import AluOpType
    bf = mybir.dt.bfloat16
    f32 = mybir.dt.float32
    i16 = mybir.dt.int16
    EXP = mybir.ActivationFunctionType.Exp

    nc = bacc.Bacc("TRN2", target_bir_lowering=False, debug=False, num_devices=NCORES)

    hsT_d = nc.declare_dram_parameter("hsT", [D, N], bf, isOutput=False)
    # packed q/k weights: [wq | wk | wqs | wks] along the output dim
    wqk_d = nc.declare_dram_parameter("wqk", [D, 256], bf, isOutput=False)
    wv_d = nc.declare_dram_parameter("wv", [D, 64], bf, isOutput=False)
    wo_d = nc.declare_dram_parameter("wo", [32, 2 * D], bf, isOutput=False)
    cos_d = nc.declare_dram_parameter("rope_cos", [128, N], bf, isOutput=False)
    sin_d = nc.declare_dram_parameter("rope_sin", [128, N], bf, isOutput=False)
    bias_d = nc.declare_dram_parameter("biases", [128, 4], f32, isOutput=False)
    ident_d = nc.declare_dram_parameter("ident", [128, 128], bf, isOutput=False)
    out_d = nc.declare_dram_parameter("outT", [D, N], f32, isOutput=True)

    NT = N // 512  # 8 q-tiles of 512
    NCP = N // 256  # 16 chunk-pairs (2x128 k rows each)
    sched = _exp_schedule()

    with tile.TileContext(nc) as tc:
        from contextlib import ExitStack

        with ExitStack() as ctx:
            const = ctx.enter_context(tc.tile_pool(name="const", bufs=1))

            hs_sb = const.tile([128, 2, N], bf, tag="hs_sb")
            wqk_sb = const.tile([128, 2, 4, 64], bf, tag="wqk_sb")
            wv_sb = const.tile([128, 2, 64], bf, tag="wv_sb")
            wo_sb = const.tile([32, 2, D], bf, tag="wo_sb")
            cos_sb = const.tile([128, N], bf, tag="cos_sb")
            sin_sb = const.tile([128, N], bf, tag="sin_sb")
            bias_sb = const.tile([128, 4], f32, tag="bias_sb")
            schraud_sb = const.tile([128, 4], f32, tag="schraud_sb")
            ident_sb = const.tile([128, 128], bf, tag="ident_sb")
            qk_sb = const.tile([128, N], bf, tag="qk_sb")
            tmp_sb = const.tile([128, N], bf, tag="tmp_sb")
            Qd = const.tile([128, N], bf, tag="Qd")
            Kd = const.tile([128, N], bf, tag="Kd")
            # v tiles: [k-chunk 128, chunk, head, 35] with [v(32) | ones(1) | pad(2)]
            v_sb = const.tile([128, 32, 2, 35], bf, tag="v_sb")

            # --- input DMAs (critical-chain first; SP issues ~650ns each) ---
            for d in range(2):
                nc.sync.dma_start(hs_sb[:, d, 0:1024],
                                  hsT_d[ts(d, 128), 0:1024])
            for d in range(2):
                nc.sync.dma_start(
                    wqk_sb[:, d, :, :],
                    wqk_d[ts(d, 128), :].rearrange("p (w x) -> p w x", w=4))
            nc.sync.dma_start(cos_sb[:, 0:1024], cos_d[:, 0:1024])
            nc.sync.dma_start(sin_sb[:, 0:1024], sin_d[:, 0:1024])
            for d in range(2):
                nc.sync.dma_start(hs_sb[:, d, 1024:N],
                                  hsT_d[ts(d, 128), 1024:N])
            nc.sync.dma_start(cos_sb[:, 1024:N], cos_d[:, 1024:N])
            nc.sync.dma_start(sin_sb[:, 1024:N], sin_d[:, 1024:N])
            nc.sync.dma_start(bias_sb[:], bias_d[:])
            for d in range(2):
                nc.sync.dma_start(wv_sb[:, d, :], wv_d[ts(d, 128), :])
            nc.sync.dma_start(
                wo_sb[:], wo_d[:].rearrange("p (h j) -> p h j", h=2))
            nc.sync.dma_start(ident_sb[:], ident_d[:])
            nc.vector.tensor_scalar(
                schraud_sb[:], bias_sb[:], SCHRAUD_A, SCHRAUD_B,
                AluOpType.mult, AluOpType.add)
            nc.vector.memset(v_sb[:, :, :, 32:33], 1.0)

            # --- phase 1: q/k projections (transposed layout) + v (natural).
            # Two projections: normal -> qk_sb and half-swapped weight
            # columns -> tmp_sb (replaces 32 partition-swap DMAs in rope).
            with tc.tile_pool(name="qkp", bufs=4, space="PSUM") as qkp:
                for t in range(NT):
                    ps = qkp.tile([128, 512], f32, tag="qkps")
                    ps2 = qkp.tile([128, 512], f32, tag="qkps")
                    for d in range(2):
                        nc.tensor.matmul(
                            ps[0:64, :], lhsT=wqk_sb[:, d, 0, :],
                            rhs=hs_sb[:, d, ts(t, 512)],
                            start=(d == 0), stop=(d == 1), tile_position=(0, 0))
                        nc.tensor.matmul(
                            ps[64:128, :], lhsT=wqk_sb[:, d, 1, :],
                            rhs=hs_sb[:, d, ts(t, 512)],
                            start=(d == 0), stop=(d == 1), tile_position=(0, 64))
                        nc.tensor.matmul(
                            ps2[0:64, :], lhsT=wqk_sb[:, d, 2, :],
                            rhs=hs_sb[:, d, ts(t, 512)],
                            start=(d == 0), stop=(d == 1), tile_position=(0, 0))
                        nc.tensor.matmul(
                            ps2[64:128, :], lhsT=wqk_sb[:, d, 3, :],
                            rhs=hs_sb[:, d, ts(t, 512)],
                            start=(d == 0), stop=(d == 1), tile_position=(0, 64))
                    nc.scalar.copy(qk_sb[:, ts(t, 512)], ps[:])
                    veng = nc.scalar if t % 2 else nc.vector
                    if veng is nc.scalar:
                        nc.scalar.copy(tmp_sb[:, ts(t, 512)], ps2[:])
                    else:
                        nc.vector.tensor_copy(tmp_sb[:, ts(t, 512)], ps2[:])
            if stage == 1:
                ob = out_d[:].bitcast(bf)
                nc.sync.dma_start(ob[0:128, 0:N], qk_sb[:])
                nc.sync.dma_start(
                    ob[128 : 128 + 128, 0 : 32 * 2 * 35],
                    v_sb[:].rearrange("p a b c -> p (a b c)"))

            # --- phase 2: partial RoPE on q and k (rows: qA qB kA kB).
            # tmp_sb already holds the half-swapped projection. cb0 must
            # precede the first scores (DVE); cb1/cb2 are emitted inside the
            # main loop (DVE, interleaved with early exps); cb3 runs on the
            # otherwise idle GpSimd engine concurrently.
            def emit_rope(sl, veng):
                # one 512-col slice (matches one projection tile)
                cs = ts(sl, 512)
                veng.tensor_mul(tmp_sb[:, cs], tmp_sb[:, cs], sin_sb[:, cs])
                veng.tensor_mul(qk_sb[:, cs], qk_sb[:, cs], cos_sb[:, cs])
                veng.tensor_add(qk_sb[:, cs], qk_sb[:, cs], tmp_sb[:, cs])
                # duplicate q and k to both halves (4-way row tiling)
                nc.sync.dma_start(Qd[0:64, cs], qk_sb[0:64, cs])
                nc.sync.dma_start(Qd[64:128, cs], qk_sb[0:64, cs])
                nc.sync.dma_start(Kd[0:64, cs], qk_sb[64:128, cs])
                nc.sync.dma_start(Kd[64:128, cs], qk_sb[64:128, cs])

            if stage >= 2:
                emit_rope(0, nc.vector)
                emit_rope(1, nc.vector)
                for sl in (4, 5, 6, 7):
                    emit_rope(sl, nc.gpsimd)
                if stage < 4:
                    emit_rope(2, nc.vector)
                    emit_rope(3, nc.vector)

            # v projection after rope emission so DVE ropes immediately
            with tc.tile_pool(name="vpp", bufs=2, space="PSUM") as vpp:
                for tv in range(32):
                    vp = vpp.tile([128, 64], f32, tag="vps")
                    for d in range(2):
                        nc.tensor.matmul(
                            vp[:], lhsT=hs_sb[:, d, ts(tv, 128)],
                            rhs=wv_sb[:, d, :],
                            start=(d == 0), stop=(d == 1))
                    veng = nc.vector if tv % 2 else nc.scalar
                    if veng is nc.scalar:
                        nc.scalar.copy(
                            v_sb[:, tv, :, 0:32],
                            vp[:].rearrange("p (h x) -> p h x", h=2))
                    else:
                        nc.vector.tensor_copy(
                            v_sb[:, tv, :, 0:32],
                            vp[:].rearrange("p (h x) -> p h x", h=2))

            if stage == 2:
                ob = out_d[:].bitcast(bf)
                nc.sync.dma_start(ob[0:128, 0:N], Qd[:])
                nc.sync.dma_start(ob[128:256, 0:N], Kd[:])

            # --- phase 3: attention main loop ---
            # step = (t, cp, h): one chunk-pair (2x128 k) x 512 q for one
            # head. Per step: 2 score matmuls (quadrant-packed) into the
            # two banks of one [128, 1024] psum tile, ONE 1024-wide exp op
            # (ACT exact or DVE Schraudolph; chunk pairs never straddle a
            # 512-block so the bias column is constant per tile), 8 PV
            # matmuls trailing pvd steps, and per-t front/tail work
            # deferred so PE never waits on the DVE chain.
            n_t = 0 if stage < 3 else (1 if stage == 3 else NT)
            with tc.tile_pool(name="ringp", bufs=3, space="PSUM") as ringp, \
                 tc.tile_pool(name="pvp", bufs=1, space="PSUM") as pvp, \
                 tc.tile_pool(name="tailp", bufs=1, space="PSUM") as tailp, \
                 tc.tile_pool(name="ptp", bufs=pvd + 5) as ptp, \
                 tc.tile_pool(name="osp", bufs=3) as osp, \
                 tc.tile_pool(name="rcp", bufs=3) as rcp, \
                 tc.tile_pool(name="ntp", bufs=8) as ntp:
                pv_tiles = {}

                def emit_scores_exp(t, cp, h):
                    sp = ringp.tile([128, 1024], f32, tag="ring",
                                    name=f"sp{t}_{cp}_{h}")
                    same = (cp // 2) == t
                    for j in range(2):
                        c = 2 * cp + j
                        g = 2 * (c % 2) + h
                        nc.tensor.matmul(
                            sp[:, ts(j, 512)],
                            lhsT=Kd[ts(g, 32), ts(c, 128)],
                            rhs=Qd[ts(g, 32), ts(t, 512)],
                            start=True, stop=True,
                            tile_position=(32 * g, 0))
                    col = 2 * h + (0 if same else 1)
                    idx = (t * NCP + cp) * 2 + h
                    eng = sched[idx]
                    if eng == "A":
                        pt = ptp.tile([128, 1024], bf, tag="pt",
                                      name=f"pt{t}_{cp}_{h}")
                        nc.scalar.activation(
                            pt[:], sp[:], EXP,
                            bias=bias_sb[:, col:col + 1], scale=1.0)
                        return pt[:]
                    pt = ptp.tile([128, 1024], i16, tag="pt",
                                  name=f"pti{t}_{cp}_{h}")
                    nc.vector.tensor_scalar(
                        pt[:], sp[:], SCHRAUD_A,
                        schraud_sb[:, col:col + 1],
                        AluOpType.mult, AluOpType.add)
                    return pt[:].bitcast(bf)

                def emit_pv(t, cp, h, pts):
                    if cp == 0 and h == 0:
                        pv_tiles[t] = pvp.tile([128, 2, 4, 64], f32, tag="pv",
                                               name=f"pv{t}")
                    pv = pv_tiles[t]
                    for j in range(2):
                        c = 2 * cp + j
                        for qs in range(4):
                            first = (cp == 0 and h == 0 and j == 0
                                     and qs == 0)
                            last = (cp == NCP - 1 and j == 1)
                            nc.tensor.matmul(
                                pv[:, h, qs, 0:33],
                                lhsT=pts[:, 512 * j + 128 * qs:
                                         512 * j + 128 * qs + 128],
                                rhs=v_sb[:, c, h, 0:33],
                                start=first, stop=last,
                                skip_group_check=not first)

                def emit_front_recip(t):
                    pv = pv_tiles[t]
                    rc = rcp.tile([128, 2, 4], f32, tag="rc", name=f"rc{t}")
                    for h in range(2):
                        nc.vector.reciprocal_approx_fast(
                            out=rc[:, h, :], in_=pv[:, h, :, 32:33])
                    O = osp.tile([128, 2, 4, 32], bf, tag="O", name=f"O{t}")
                    return (t, pv, rc, O)

                def emit_front_norm(arg):
                    t, pv, rc, O, h = arg
                    for qs in range(4):
                        nc.vector.tensor_scalar(
                            O[:, h, qs, :], pv[:, h, qs, 0:32],
                            rc[:, h, qs:qs + 1], None, AluOpType.mult)

                def emit_tail_a(tail):
                    t, O = tail
                    tp = tailp.tile([32, 8, 128], bf, tag="tail",
                                    name=f"tp{t}")
                    numt = ntp.tile([32, 2, 512], bf, tag="numt",
                                    name=f"numt{t}")
                    for h in range(2):
                        for qs in range(4):
                            first = (h == 0 and qs == 0)
                            nc.tensor.matmul(
                                tp[:, 4 * h + qs, :], lhsT=O[:, h, qs, :],
                                rhs=ident_sb[:], is_transpose=True,
                                start=first, stop=True,
                                skip_group_check=not first)
                        nc.vector.tensor_copy(
                            numt[:, h, :],
                            tp[:, 4 * h:4 * h + 4, :].rearrange(
                                "p a b -> p (a b)"))
                    return (t, numt)

                def emit_tail_b(arg):
                    t, numt, jc = arg
                    op_ps = tailp.tile([128, 512], f32, tag="tail",
                                       name=f"op{t}_{jc}")
                    for h in range(2):
                        nc.tensor.matmul(
                            op_ps[:], lhsT=wo_sb[:, h, ts(jc, 128)],
                            rhs=numt[:, h, :],
                            start=(h == 0), stop=(h == 1))
                    op_sb = ntp.tile([128, 512], f32, tag="opsb",
                                     name=f"opsb{t}_{jc}")
                    nc.scalar.copy(op_sb[:], op_ps[:])
                    nc.sync.dma_start(
                        out_d[ts(jc, 128), ts(t, 512)], op_sb[:])

                from collections import deque
                pend_pv = deque()    # (t, c, pts), emitted pvd steps later
                todo = {}            # step_idx -> list of thunks
                step = 0

                def after(delay, fn, arg):
                    todo.setdefault(step + delay, []).append((fn, arg))

                def run_due():
                    for fn, arg in todo.pop(step, []):
                        fn(arg)

                def _tail_a(arg):
                    t, O, d0 = arg
                    t, numt = emit_tail_a((t, O))
                    after(d0, emit_tail_b, (t, numt, 0))
                    after(d0 + 1, emit_tail_b, (t, numt, 1))

                def pop_pv():
                    t_, cp_, h_, pp_ = pend_pv.popleft()
                    emit_pv(t_, cp_, h_, pp_)
                    if cp_ == NCP - 1 and h_ == 1:
                        # recip+norm immediately (DVE was kept free of exp
                        # work for this chunk-pair) so the single pv bank
                        # frees before pv(t+1) needs it; PE-side tail ops
                        # follow a few steps later off the scores ring
                        fast = (t_ == n_t - 1)
                        t2, pv2, rc2, O2 = emit_front_recip(t_)
                        emit_front_norm((t2, pv2, rc2, O2, 0))
                        emit_front_norm((t2, pv2, rc2, O2, 1))
                        if fast:
                            after(1, _tail_a, (t2, O2, 1))
                        else:
                            after(4, _tail_a, (t2, O2, 2))

                for t in range(n_t):
                    pvd_eff = 1 if t == n_t - 1 else pvd
                    for cp in range(NCP):
                        for h in range(2):
                            pend_pv.append(
                                (t, cp, h, emit_scores_exp(t, cp, h)))
                            while len(pend_pv) > pvd_eff:
                                pop_pv()
                            run_due()
                            step += 1
                            if stage >= 4 and t == 0 and h == 1 \
                                    and cp in (1, 3):
                                emit_rope(2 if cp == 1 else 3, nc.vector)
                while pend_pv:
                    pop_pv()
                while todo:
                    run_due()
                    step += 1
    nc.compile()
    return nc


def _rope_tables():
    j = np.arange(8, dtype=np.float64)
    inv = 10000.0 ** (-(2.0 * j / HD))  # [8]
    ang = np.arange(N, dtype=np.float64)[None, :] * inv[:, None]  # [8, N]
    cosb = np.ones((32, N), dtype=np.float64)
    sinb = np.zeros((32, N), dtype=np.float64)
    cosb[0:8] = np.cos(ang)
    cosb[16:24] = np.cos(ang)
    sinb[0:8] = -np.sin(ang)
    sinb[16:24] = np.sin(ang)
    cos128 = np.tile(cosb, (4, 1)).astype(BF16)
    sin128 = np.tile(sinb, (4, 1)).astype(BF16)
    return cos128, sin128


def kernel(**inputs):
    hs = np.asarray(inputs["hidden_states"], dtype=np.float32)
    qw = np.asarray(inputs["q_w"], dtype=np.float32)
    kw = np.asarray(inputs["k_w"], dtype=np.float32)
    vw = np.asarray(inputs["v_w"], dtype=np.float32)
    ow = np.asarray(inputs["o_w"], dtype=np.float32)
    ob = np.asarray(inputs["o_b"], dtype=np.float32)
    qb = np.asarray(inputs["q_b"], dtype=np.float32)
    kb = np.asarray(inputs["k_b"], dtype=np.float32)
    vb = np.asarray(inputs["v_b"], dtype=np.float32)
    ab = np.asarray(inputs["attention_biases"], dtype=np.float32)
    seq = int(np.asarray(inputs["sequence_length"]))
    assert seq == SEQ, f"kernel compiled for sequence_length={SEQ}, got {seq}"
    assert hs.shape == (B, N, D)
    assert not (np.any(qb) or np.any(kb) or np.any(vb)), "nonzero qkv bias unsupported"

    stage = int(os.environ.get("KERNEL_STAGE", "4"))
    if ("nc", stage) not in _NC_CACHE:
        _NC_CACHE[("nc", stage)] = _build_nc(stage)
    nc = _NC_CACHE[("nc", stage)]

    cos128, sin128 = _rope_tables()
    ident = np.eye(128, dtype=np.float32).astype(BF16)
    # half-swap of each head's 32 output dims (partner rows for rope)
    perm = np.concatenate([np.arange(16, 32), np.arange(16)])
    perm64 = np.concatenate([perm, perm + 32])
    in_maps = []
    for c in range(NCORES):
        b = c // 4
        h0 = 2 * (c % 4)
        rows = slice(h0 * HD, h0 * HD + 2 * HD)
        qwr = qw[rows, :] * SCALE
        kwr = kw[rows, :]
        wqk = np.concatenate(
            [qwr.T, kwr.T, qwr[perm64].T, kwr[perm64].T], axis=1)
        in_maps.append({
            "hsT": np.ascontiguousarray(hs[b].T).astype(BF16),
            "wqk": np.ascontiguousarray(wqk).astype(BF16),
            "wv": np.ascontiguousarray(vw[rows, :].T).astype(BF16),
            "wo": np.ascontiguousarray(
                ow[:, rows].T.reshape(2, 32, D).transpose(1, 0, 2)
                .reshape(32, 2 * D)).astype(BF16),
            "rope_cos": cos128,
            "rope_sin": sin128,
            "ident": ident,
            "biases": np.ascontiguousarray(
                np.broadcast_to(ab[h0:h0 + 2].reshape(1, 4), (128, 4))
            ).astype(np.float32),
        })

    global _LAST_IN_MAPS, _LAST_RESULTS
    _LAST_IN_MAPS = in_maps
    from concourse.bass_utils import run_bass_kernel_spmd
    res = run_bass_kernel_spmd(nc, in_maps, core_ids=list(range(NCORES)))
    _LAST_RESULTS = res.results
    out = np.zeros((B, N, D), dtype=np.float32)
    for c in range(NCORES):
        out[c // 4] += res.results[c]["outT"].T.astype(np.float32)
    out += ob[None, None, :]
    return out


# revision 64
# speedup vs baseline: 1.3699x; 1.0817x over previous
"""AnyVariateAttention Trainium2 kernel (8 NeuronCores, SPMD).

Sharding: 16 (batch, head) pairs / 8 cores -> each core computes 2 adjacent
heads of one batch (core c: batch c//4, heads 2*(c%4), 2*(c%4)+1).

Per core: QKV projection (transposed layouts), partial RoPE, flash-style
attention with transposed scores (S^T tiles [k,q]), block-bias folded into
the exp (ACT bias port / Schraudolph bias), softmax denominator via a
ones-column appended to V.

v2 restructure vs baseline:
- PV uses P^T as the *stationary* operand ([128k x 128q]) and V|ones as the
  *moving* operand (33 wide): 33 cycles/chunk instead of 512 -> ~4x less PE
  time in PV.
- PV output lands as O [q, hd]; normalization is a per-partition
  reciprocal+scale fused into the PSUM->SBUF eviction, then O is transposed
  back to [hd, q] on the PE (identity-matmul transpose) for the out-proj.
- exp is spread across ACT (exact), DVE and GpSimd/Pool (Schraudolph
  bit-trick) with a static greedy schedule balancing per-engine load.
- PSUM: one 3-slot ring of 4KB slots (score tiles / transpose tile / oproj
  tiles) + a 2-slot pool of per-t PV accumulator banks (both heads x 4
  q-slices in one 2KB bank; single start=True marks the bank, the other
  accumulation groups use start=False + skip_group_check).
"""

import sys
import os
import numpy as np

for _p in ("/opt/trn_rl_repo",):
    if _p not in sys.path:
        sys.path.insert(0, _p)

import ml_dtypes

BF16 = ml_dtypes.bfloat16

B, N, D, H, HD = 2, 4096, 256, 8, 32
SEQ = 512
SCALE = HD ** -0.5
NCORES = 8
SCHRAUD_A = 184.6650390625  # 128 * log2(e)
SCHRAUD_B = 16256.0 - 7.4

_NC_CACHE = {}


def _exp_schedule():
    """Static (t, cp, h) -> engine schedule for the 256 exp tiles
    ([128, 1024] each). GPSIMD cannot read PSUM, so only ACT (exact exp)
    and DVE (Schraudolph) can drain the score banks.

    Greedy finish-time balancing with cost-model estimates (ns): per-tile
    busy ACT 1038 / DVE 1192; fixed loads ACT ~20us (qk/v/op evictions),
    DVE ~25us (recip, norm, numt evict, rope cb0)."""
    costs = {"A": 1038.0, "D": 1192.0}
    load = {"A": 25000.0, "D": 29500.0}
    sched = []
    for i in range(256):
        cp = (i // 2) % 16
        if cp == 15 and i % 2 == 1:
            # keep DVE free right when each q-tile's recip+norm chain lands
            e = "A"
        else:
            e = min("AD", key=lambda e: load[e] + costs[e])
        sched.append(e)
        load[e] += costs[e]
    return sched


def _build_nc(stage=4, pvd=8):
    import concourse.bass as bass
    import concourse.tile as tile
    from concourse import bacc, mybir
    from concourse.bass import ts

    from concourse.alu_op_type import AluOpType
    bf = mybir.dt.bfloat16
    f32 = mybir.dt.float32
    i16 = mybir.dt.int16
    EXP = mybir.ActivationFunctionType.Exp

    nc = bacc.Bacc("TRN2", target_bir_lowering=False, debug=False, num_devices=NCORES)

    hsT_d = nc.declare_dram_parameter("hsT", [D, N], bf, isOutput=False)
    # packed q/k weights: [wq | wk | wqs | wks] along the output dim
    wqk_d = nc.declare_dram_parameter("wqk", [D, 256], bf, isOutput=False)
    wv_d = nc.declare_dram_parameter("wv", [D, 64], bf, isOutput=False)
    wo_d = nc.declare_dram_parameter("wo", [32, 2 * D], bf, isOutput=False)
    cos_d = nc.declare_dram_parameter("rope_cos", [128, N], bf, isOutput=False)
    sin_d = nc.declare_dram_parameter("rope_sin", [128, N], bf, isOutput=False)
    bias_d = nc.declare_dram_parameter("biases", [128, 4], f32, isOutput=False)
    ident_d = nc.declare_dram_parameter("ident", [128, 128], bf, isOutput=False)
    out_d = nc.declare_dram_parameter("outT", [D, N], f32, isOutput=True)

    NT = N // 512  # 8 q-tiles of 512
    NCP = N // 256  # 16 chunk-pairs (2x128 k rows each)
    sched = _exp_schedule()

    with tile.TileContext(nc) as tc:
        from contextlib import ExitStack

        with ExitStack() as ctx:
            const = ctx.enter_context(tc.tile_pool(name="const", bufs=1))

            hs_sb = const.tile([128, 2, N], bf, tag="hs_sb")
            wqk_sb = const.tile([128, 2, 4, 64], bf, tag="wqk_sb")
            wv_sb = const.tile([128, 2, 64], bf, tag="wv_sb")
            wo_sb = const.tile([32, 2, D], bf, tag="wo_sb")
            cos_sb = const.tile([128, N], bf, tag="cos_sb")
            sin_sb = const.tile([128, N], bf, tag="sin_sb")
            bias_sb = const.tile([128, 4], f32, tag="bias_sb")
            schraud_sb = const.tile([128, 4], f32, tag="schraud_sb")
            ident_sb = const.tile([128, 128], bf, tag="ident_sb")
            qk_sb = const.tile([128, N], bf, tag="qk_sb")
            tmp_sb = const.tile([128, N], bf, tag="tmp_sb")
            Qd = const.tile([128, N], bf, tag="Qd")
            Kd = const.tile([128, N], bf, tag="Kd")
            # v tiles: [k-chunk 128, chunk, head, 35] with [v(32) | ones(1) | pad(2)]
            v_sb = const.tile([128, 32, 2, 35], bf, tag="v_sb")

            # --- input DMAs (critical-chain first; SP issues ~650ns each) ---
            for d in range(2):
                nc.sync.dma_start(
                    wqk_sb[:, d, :, :],
                    wqk_d[ts(d, 128), :].rearrange("p (w x) -> p w x", w=4))
            for d in range(2):
                nc.sync.dma_start(hs_sb[:, d, 0:512],
                                  hsT_d[ts(d, 128), 0:512])
            for d in range(2):
                nc.sync.dma_start(hs_sb[:, d, 512:1024],
                                  hsT_d[ts(d, 128), 512:1024])
            nc.sync.dma_start(cos_sb[:, 0:1024], cos_d[:, 0:1024])
            nc.sync.dma_start(sin_sb[:, 0:1024], sin_d[:, 0:1024])
            for d in range(2):
                nc.sync.dma_start(hs_sb[:, d, 1024:3072],
                                  hsT_d[ts(d, 128), 1024:3072])
            for d in range(2):
                nc.sync.dma_start(hs_sb[:, d, 3072:N],
                                  hsT_d[ts(d, 128), 3072:N])
            nc.sync.dma_start(cos_sb[:, 1024:N], cos_d[:, 1024:N])
            nc.sync.dma_start(sin_sb[:, 1024:N], sin_d[:, 1024:N])
            nc.sync.dma_start(bias_sb[:], bias_d[:])
            for d in range(2):
                nc.sync.dma_start(wv_sb[:, d, :], wv_d[ts(d, 128), :])
            nc.sync.dma_start(
                wo_sb[:], wo_d[:].rearrange("p (h j) -> p h j", h=2))
            nc.sync.dma_start(ident_sb[:], ident_d[:])
            nc.vector.tensor_scalar(
                schraud_sb[:], bias_sb[:], SCHRAUD_A, SCHRAUD_B,
                AluOpType.mult, AluOpType.add)
            nc.vector.memset(v_sb[:, :, :, 32:33], 1.0)

            # --- phase 3: attention main loop ---
            # step = (t, cp, h): one chunk-pair (2x128 k) x 512 q for one
            # head. Per step: 2 score matmuls (quadrant-packed) into the
            # two banks of one [128, 1024] psum tile, ONE 1024-wide exp op
            # (ACT exact or DVE Schraudolph; chunk pairs never straddle a
            # 512-block so the bias column is constant per tile), 8 PV
            # matmuls trailing pvd steps, and per-t front/tail work
            # deferred so PE never waits on the DVE chain.
            n_t = 0 if stage < 3 else (1 if stage == 3 else NT)
            with tc.tile_pool(name="ringp", bufs=3, space="PSUM") as ringp, \
                 tc.tile_pool(name="pvp", bufs=1, space="PSUM") as pvp, \
                 tc.tile_pool(name="tailp", bufs=1, space="PSUM") as tailp, \
                 tc.tile_pool(name="ptp", bufs=pvd + 5) as ptp, \
                 tc.tile_pool(name="osp", bufs=3) as osp, \
                 tc.tile_pool(name="rcp", bufs=3) as rcp, \
                 tc.tile_pool(name="ntp", bufs=8) as ntp:
                pv_tiles = {}

                # q/k dual projection (normal + half-swapped weight columns,
                # replacing rope's partition-swap DMAs) through the psum ring
                def emit_qkproj(t):
                    pr = ringp.tile([128, 1024], f32, tag="ring",
                                    name=f"qkproj{t}")
                    for d in range(2):
                        for w in range(4):
                            nc.tensor.matmul(
                                pr[64 * (w % 2):64 * (w % 2) + 64,
                                   512 * (w // 2):512 * (w // 2) + 512],
                                lhsT=wqk_sb[:, d, w, :],
                                rhs=hs_sb[:, d, ts(t, 512)],
                                start=(d == 0), stop=(d == 1),
                                tile_position=(0, 64 * (w % 2)))
                    nc.scalar.copy(qk_sb[:, ts(t, 512)], pr[:, 0:512])
                    nc.vector.tensor_copy(tmp_sb[:, ts(t, 512)],
                                          pr[:, 512:1024])

                # one 512-col rope slice (matches one projection tile)
                def emit_rope(sl, veng):
                    cs = ts(sl, 512)
                    veng.tensor_mul(tmp_sb[:, cs], tmp_sb[:, cs],
                                    sin_sb[:, cs])
                    veng.tensor_mul(qk_sb[:, cs], qk_sb[:, cs],
                                    cos_sb[:, cs])
                    veng.tensor_add(qk_sb[:, cs], qk_sb[:, cs],
                                    tmp_sb[:, cs])
                    # duplicate q and k to both halves (4-way row tiling)
                    nc.sync.dma_start(Qd[0:64, cs], qk_sb[0:64, cs])
                    nc.sync.dma_start(Qd[64:128, cs], qk_sb[0:64, cs])
                    nc.sync.dma_start(Kd[0:64, cs], qk_sb[64:128, cs])
                    nc.sync.dma_start(Kd[64:128, cs], qk_sb[64:128, cs])

                # v projection for chunks [lo, lo+n): n*64 f32 columns of a
                # ring tile; one start=True per psum bank, evicts batched 4
                def emit_vproj(vi, lo, n):
                    vt = ringp.tile([128, 16, 64], f32, tag="ring",
                                    name=f"vproj{vi}")
                    for j in range(n):
                        bf_ = (j % 8 == 0)
                        for d in range(2):
                            nc.tensor.matmul(
                                vt[:, j, :], lhsT=hs_sb[:, d, ts(lo + j, 128)],
                                rhs=wv_sb[:, d, :],
                                start=(d == 0 and bf_), stop=(d == 1),
                                skip_group_check=not bf_)
                    for g in range(0, n, 4):
                        src = vt[:, g:g + 4, :].rearrange(
                            "p a (h x) -> p a h x", h=2)
                        if (g // 4) % 2:
                            nc.scalar.copy(v_sb[:, lo + g:lo + g + 4, :, 0:32],
                                           src)
                        else:
                            nc.vector.tensor_copy(
                                v_sb[:, lo + g:lo + g + 4, :, 0:32], src)

                # pre-loop: enough projections/rope for the first ~8 steps;
                # slices 4,5 rope on the idle GpSimd engine
                for t_ in (0, 1, 2, 4, 5):
                    emit_qkproj(t_)
                for sl in (0, 1, 2):
                    emit_rope(sl, nc.vector)
                for sl in (4, 5):
                    emit_rope(sl, nc.gpsimd)
                emit_vproj(0, 0, 8)
                if stage < 4:
                    for t_ in (3, 6, 7):
                        emit_qkproj(t_)
                    for sl in (3, 6, 7):
                        emit_rope(sl, nc.vector)
                    emit_vproj(1, 8, 16)
                    emit_vproj(2, 24, 8)
                if stage == 1:
                    ob = out_d[:].bitcast(bf)
                    nc.sync.dma_start(ob[0:128, 0:N], qk_sb[:])
                    nc.sync.dma_start(
                        ob[128:256, 0:32 * 2 * 35],
                        v_sb[:].rearrange("p a b c -> p (a b c)"))
                if stage == 2:
                    ob = out_d[:].bitcast(bf)
                    nc.sync.dma_start(ob[0:128, 0:N], Qd[:])
                    nc.sync.dma_start(ob[128:256, 0:N], Kd[:])

                def emit_scores_exp(t, cp, h):
                    sp = ringp.tile([128, 1024], f32, tag="ring",
                                    name=f"sp{t}_{cp}_{h}")
                    same = (cp // 2) == t
                    for j in range(2):
                        c = 2 * cp + j
                        g = 2 * (c % 2) + h
                        nc.tensor.matmul(
                            sp[:, ts(j, 512)],
                            lhsT=Kd[ts(g, 32), ts(c, 128)],
                            rhs=Qd[ts(g, 32), ts(t, 512)],
                            start=True, stop=True,
                            tile_position=(32 * g, 0))
                    col = 2 * h + (0 if same else 1)
                    idx = (t * NCP + cp) * 2 + h
                    eng = sched[idx]
                    if eng == "A":
                        pt = ptp.tile([128, 1024], bf, tag="pt",
                                      name=f"pt{t}_{cp}_{h}")
                        nc.scalar.activation(
                            pt[:], sp[:], EXP,
                            bias=bias_sb[:, col:col + 1], scale=1.0)
                        return pt[:]
                    pt = ptp.tile([128, 1024], i16, tag="pt",
                                  name=f"pti{t}_{cp}_{h}")
                    nc.vector.tensor_scalar(
                        pt[:], sp[:], SCHRAUD_A,
                        schraud_sb[:, col:col + 1],
                        AluOpType.mult, AluOpType.add)
                    return pt[:].bitcast(bf)

                def emit_pv(t, cp, h, pts):
                    if cp == 0 and h == 0:
                        pv_tiles[t] = pvp.tile([128, 2, 4, 64], f32, tag="pv",
                                               name=f"pv{t}")
                    pv = pv_tiles[t]
                    for j in range(2):
                        c = 2 * cp + j
                        for qs in range(4):
                            first = (cp == 0 and h == 0 and j == 0
                                     and qs == 0)
                            last = (cp == NCP - 1 and j == 1)
                            nc.tensor.matmul(
                                pv[:, h, qs, 0:33],
                                lhsT=pts[:, 512 * j + 128 * qs:
                                         512 * j + 128 * qs + 128],
                                rhs=v_sb[:, c, h, 0:33],
                                start=first, stop=last,
                                skip_group_check=not first)

                def emit_front_recip(t):
                    pv = pv_tiles[t]
                    rc = rcp.tile([128, 2, 4], f32, tag="rc", name=f"rc{t}")
                    for h in range(2):
                        nc.vector.reciprocal_approx_fast(
                            out=rc[:, h, :], in_=pv[:, h, :, 32:33])
                    O = osp.tile([128, 2, 4, 32], bf, tag="O", name=f"O{t}")
                    return (t, pv, rc, O)

                def emit_front_norm(arg):
                    t, pv, rc, O, h = arg
                    for qs in range(4):
                        nc.vector.tensor_scalar(
                            O[:, h, qs, :], pv[:, h, qs, 0:32],
                            rc[:, h, qs:qs + 1], None, AluOpType.mult)

                def emit_tail_a(tail):
                    t, O = tail
                    tp = tailp.tile([32, 8, 128], bf, tag="tail",
                                    name=f"tp{t}")
                    numt = ntp.tile([32, 2, 512], bf, tag="numt",
                                    name=f"numt{t}")
                    for h in range(2):
                        for qs in range(4):
                            first = (h == 0 and qs == 0)
                            nc.tensor.matmul(
                                tp[:, 4 * h + qs, :], lhsT=O[:, h, qs, :],
                                rhs=ident_sb[:], is_transpose=True,
                                start=first, stop=True,
                                skip_group_check=not first)
                        nc.vector.tensor_copy(
                            numt[:, h, :],
                            tp[:, 4 * h:4 * h + 4, :].rearrange(
                                "p a b -> p (a b)"))
                    return (t, numt)

                def emit_tail_b(arg):
                    t, numt, jc = arg
                    op_ps = tailp.tile([128, 512], f32, tag="tail",
                                       name=f"op{t}_{jc}")
                    for h in range(2):
                        nc.tensor.matmul(
                            op_ps[:], lhsT=wo_sb[:, h, ts(jc, 128)],
                            rhs=numt[:, h, :],
                            start=(h == 0), stop=(h == 1))
                    op_sb = ntp.tile([128, 512], f32, tag="opsb",
                                     name=f"opsb{t}_{jc}")
                    nc.scalar.copy(op_sb[:], op_ps[:])
                    nc.sync.dma_start(
                        out_d[ts(jc, 128), ts(t, 512)], op_sb[:])

                from collections import deque
                pend_pv = deque()    # (t, c, pts), emitted pvd steps later
                todo = {}            # step_idx -> list of thunks
                step = 0

                def after(delay, fn, arg):
                    todo.setdefault(step + delay, []).append((fn, arg))

                def run_due():
                    for fn, arg in todo.pop(step, []):
                        fn(arg)

                def _tail_a(arg):
                    t, O, d0 = arg
                    t, numt = emit_tail_a((t, O))
                    after(d0, emit_tail_b, (t, numt, 0))
                    after(d0 + 1, emit_tail_b, (t, numt, 1))

                fast_state = {}

                def pop_pv():
                    t_, cp_, h_, pp_ = pend_pv.popleft()
                    emit_pv(t_, cp_, h_, pp_)
                    fast = (t_ == n_t - 1)
                    if cp_ == NCP - 1 and fast and n_t > 1:
                        # last q-tile: fine-grained per-head drain so the
                        # recip/norm/transpose/evict chains of the two heads
                        # overlap instead of serializing at the very end
                        pv = pv_tiles[t_]
                        if h_ == 0:
                            rc = rcp.tile([128, 2, 4], f32, tag="rc",
                                          name=f"rc{t_}")
                            O = osp.tile([128, 2, 4, 32], bf, tag="O",
                                         name=f"O{t_}")
                            tp = tailp.tile([32, 8, 128], bf, tag="tail",
                                            name=f"tp{t_}")
                            numt = ntp.tile([32, 2, 512], bf, tag="numt",
                                            name=f"numt{t_}")
                            fast_state.update(rc=rc, O=O, tp=tp, numt=numt)
                        else:
                            rc, O = fast_state["rc"], fast_state["O"]
                            tp, numt = fast_state["tp"], fast_state["numt"]
                        nc.vector.reciprocal_approx_fast(
                            out=rc[:, h_, :], in_=pv[:, h_, :, 32:33])
                        for qs in range(4):
                            nc.vector.tensor_scalar(
                                O[:, h_, qs, :], pv[:, h_, qs, 0:32],
                                rc[:, h_, qs:qs + 1], None, AluOpType.mult)
                        for qs in range(4):
                            first = (h_ == 0 and qs == 0)
                            nc.tensor.matmul(
                                tp[:, 4 * h_ + qs, :], lhsT=O[:, h_, qs, :],
                                rhs=ident_sb[:], is_transpose=True,
                                start=first, stop=True,
                                skip_group_check=not first)
                        nc.vector.tensor_copy(
                            numt[:, h_, :],
                            tp[:, 4 * h_:4 * h_ + 4, :].rearrange(
                                "p a b -> p (a b)"))
                        if h_ == 1:
                            emit_tail_b((t_, numt, 0))
                            emit_tail_b((t_, numt, 1))
                        return
                    if cp_ == NCP - 1 and h_ == 1:
                        # recip+norm immediately (DVE was kept free of exp
                        # work for this chunk-pair) so the single pv bank
                        # frees before pv(t+1) needs it; PE-side tail ops
                        # follow a few steps later off the scores ring
                        t2, pv2, rc2, O2 = emit_front_recip(t_)
                        emit_front_norm((t2, pv2, rc2, O2, 0))
                        emit_front_norm((t2, pv2, rc2, O2, 1))
                        if t_ == n_t - 1:
                            after(1, _tail_a, (t2, O2, 1))
                        else:
                            after(6, _tail_a, (t2, O2, 2))

                for t in range(n_t):
                    pvd_eff = 1 if t == n_t - 1 else pvd
                    for cp in range(NCP):
                        for h in range(2):
                            pend_pv.append(
                                (t, cp, h, emit_scores_exp(t, cp, h)))
                            # give the head PV 2 extra steps when it opens a
                            # new q-tile (waits on the pv bank being freed by
                            # the previous tile's norm chain)
                            lim = pvd_eff
                            if pend_pv and pend_pv[0][1] == 0 \
                                    and pend_pv[0][0] > 0:
                                lim = pvd_eff + 4
                            while len(pend_pv) > lim:
                                pop_pv()
                            run_due()
                            step += 1
                            if stage >= 4 and t == 0 and h == 1:
                                if cp == 0:
                                    emit_qkproj(3)
                                elif cp == 1:
                                    emit_rope(3, nc.vector)
                                    emit_vproj(1, 8, 16)
                                elif cp == 2:
                                    emit_qkproj(6)
                                elif cp == 4:
                                    emit_rope(6, nc.vector)
                                elif cp == 5:
                                    emit_qkproj(7)
                                    emit_vproj(2, 24, 8)
                                elif cp == 7:
                                    emit_rope(7, nc.vector)
                while pend_pv:
                    pop_pv()
                while todo:
                    run_due()
                    step += 1
    nc.compile()
    return nc


def _rope_tables():
    j = np.arange(8, dtype=np.float64)
    inv = 10000.0 ** (-(2.0 * j / HD))  # [8]
    ang = np.arange(N, dtype=np.float64)[None, :] * inv[:, None]  # [8, N]
    cosb = np.ones((32, N), dtype=np.float64)
    sinb = np.zeros((32, N), dtype=np.float64)
    cosb[0:8] = np.cos(ang)
    cosb[16:24] = np.cos(ang)
    sinb[0:8] = -np.sin(ang)
    sinb[16:24] = np.sin(ang)
    cos128 = np.tile(cosb, (4, 1)).astype(BF16)
    sin128 = np.tile(sinb, (4, 1)).astype(BF16)
    return cos128, sin128


def kernel(**inputs):
    hs = np.asarray(inputs["hidden_states"], dtype=np.float32)
    qw = np.asarray(inputs["q_w"], dtype=np.float32)
    kw = np.asarray(inputs["k_w"], dtype=np.float32)
    vw = np.asarray(inputs["v_w"], dtype=np.float32)
    ow = np.asarray(inputs["o_w"], dtype=np.float32)
    ob = np.asarray(inputs["o_b"], dtype=np.float32)
    qb = np.asarray(inputs["q_b"], dtype=np.float32)
    kb = np.asarray(inputs["k_b"], dtype=np.float32)
    vb = np.asarray(inputs["v_b"], dtype=np.float32)
    ab = np.asarray(inputs["attention_biases"], dtype=np.float32)
    seq = int(np.asarray(inputs["sequence_length"]))
    assert seq == SEQ, f"kernel compiled for sequence_length={SEQ}, got {seq}"
    assert hs.shape == (B, N, D)
    assert not (np.any(qb) or np.any(kb) or np.any(vb)), "nonzero qkv bias unsupported"

    stage = int(os.environ.get("KERNEL_STAGE", "4"))
    if ("nc", stage) not in _NC_CACHE:
        _NC_CACHE[("nc", stage)] = _build_nc(stage)
    nc = _NC_CACHE[("nc", stage)]

    cos128, sin128 = _rope_tables()
    ident = np.eye(128, dtype=np.float32).astype(BF16)
    # half-swap of each head's 32 output dims (partner rows for rope)
    perm = np.concatenate([np.arange(16, 32), np.arange(16)])
    perm64 = np.concatenate([perm, perm + 32])
    in_maps = []
    for c in range(NCORES):
        b = c // 4
        h0 = 2 * (c % 4)
        rows = slice(h0 * HD, h0 * HD + 2 * HD)
        qwr = qw[rows, :] * SCALE
        kwr = kw[rows, :]
        wqk = np.concatenate(
            [qwr.T, kwr.T, qwr[perm64].T, kwr[perm64].T], axis=1)
        in_maps.append({
            "hsT": np.ascontiguousarray(hs[b].T).astype(BF16),
            "wqk": np.ascontiguousarray(wqk).astype(BF16),
            "wv": np.ascontiguousarray(vw[rows, :].T).astype(BF16),
            "wo": np.ascontiguousarray(
                ow[:, rows].T.reshape(2, 32, D).transpose(1, 0, 2)
                .reshape(32, 2 * D)).astype(BF16),
            "rope_cos": cos128,
            "rope_sin": sin128,
            "ident": ident,
            "biases": np.ascontiguousarray(
                np.broadcast_to(ab[h0:h0 + 2].reshape(1, 4), (128, 4))
            ).astype(np.float32),
        })

    global _LAST_IN_MAPS, _LAST_RESULTS
    _LAST_IN_MAPS = in_maps
    from concourse.bass_utils import run_bass_kernel_spmd
    res = run_bass_kernel_spmd(nc, in_maps, core_ids=list(range(NCORES)))
    _LAST_RESULTS = res.results
    out = np.zeros((B, N, D), dtype=np.float32)
    for c in range(NCORES):
        out[c // 4] += res.results[c]["outT"].T.astype(np.float32)
    out += ob[None, None, :]
    return out


# revision 67
# speedup vs baseline: 1.3807x; 1.0079x over previous
"""AnyVariateAttention Trainium2 kernel (8 NeuronCores, SPMD).

Sharding: 16 (batch, head) pairs / 8 cores -> each core computes 2 adjacent
heads of one batch (core c: batch c//4, heads 2*(c%4), 2*(c%4)+1).

Per core: QKV projection (transposed layouts), partial RoPE, flash-style
attention with transposed scores (S^T tiles [k,q]), block-bias folded into
the exp (ACT bias port / Schraudolph bias), softmax denominator via a
ones-column appended to V.

v2 restructure vs baseline (312us -> ~228us):
- PV uses P^T as the *stationary* operand ([128k x 128q]) and V|ones as the
  *moving* operand (33 wide): 33 cycles/chunk instead of 512 -> ~4x less PE
  time in PV. PV output lands as O [q, hd]; normalization is a
  per-partition reciprocal+scale fused into the PSUM->SBUF eviction, then O
  is transposed back to [hd, q] on the PE (identity-matmul transpose) for
  the out-proj.
- The softmax exp is the real bottleneck: 33.5M score elements/core must
  drain PSUM->SBUF through ACT (exact exp, ~1.04us/tile) and DVE
  (Schraudolph bit-trick, ~1.19us/tile) only — the GPSIMD/Pool engine
  cannot access PSUM (BIR verifier rule). A static greedy schedule
  balances the 256 [128,1024] exp tiles across both engines; steps are
  (t, chunk-pair, head) so each tile shares one bias column.
- RoPE swap-halves are produced by a second q/k projection with
  host-side half-swapped weight columns (no partition-swap DMAs); rope
  runs in 512-col slices on DVE (first slices) and Pool, interleaved with
  the early main loop, and projections flow lazily through the psum ring.
- PSUM (8 banks): 3-slot ring of [128,1024] score tiles (also carries
  lazy projections), 1 bank of PV accumulators (2 heads x 4 q-slices x
  [33] in one 2KB bank; single start=True marks the bank, later
  accumulation groups use start=False + skip_group_check), 1 bank for
  tail tiles (transpose + out-proj).
- Per-q-tile tail (recip/norm on DVE, PE transposes + out-proj) is
  deferred several steps and the last tile drains with per-head
  interleaving to shorten the endgame.
"""

import sys
import os
import numpy as np

for _p in ("/opt/trn_rl_repo",):
    if _p not in sys.path:
        sys.path.insert(0, _p)

import ml_dtypes

BF16 = ml_dtypes.bfloat16

B, N, D, H, HD = 2, 4096, 256, 8, 32
SEQ = 512
SCALE = HD ** -0.5
NCORES = 8
SCHRAUD_A = 184.6650390625  # 128 * log2(e)
SCHRAUD_B = 16256.0 - 7.4

_NC_CACHE = {}


def _exp_schedule():
    """Static (t, cp, h) -> engine schedule for the 256 exp tiles
    ([128, 1024] each). GPSIMD cannot read PSUM, so only ACT (exact exp)
    and DVE (Schraudolph) can drain the score banks.

    Greedy finish-time balancing with cost-model estimates (ns): per-tile
    busy ACT 1038 / DVE 1192; fixed loads ACT ~20us (qk/v/op evictions),
    DVE ~25us (recip, norm, numt evict, rope cb0)."""
    costs = {"A": 1038.0, "D": 1192.0}
    load = {"A": 25000.0, "D": 29500.0}
    sched = []
    for i in range(256):
        cp = (i // 2) % 16
        e = min("AD", key=lambda e: load[e] + costs[e])
        sched.append(e)
        load[e] += costs[e]
    return sched


def _build_nc(stage=4, pvd=8):
    import concourse.bass as bass
    import concourse.tile as tile
    from concourse import bacc, mybir
    from concourse.bass import ts

    from concourse.alu_op_type import AluOpType
    bf = mybir.dt.bfloat16
    f32 = mybir.dt.float32
    i16 = mybir.dt.int16
    EXP = mybir.ActivationFunctionType.Exp

    nc = bacc.Bacc("TRN2", target_bir_lowering=False, debug=False, num_devices=NCORES)

    hsT_d = nc.declare_dram_parameter("hsT", [D, N], bf, isOutput=False)
    # packed q/k weights: [wq | wk | wqs | wks] along the output dim
    wqk_d = nc.declare_dram_parameter("wqk", [D, 256], bf, isOutput=False)
    wv_d = nc.declare_dram_parameter("wv", [D, 64], bf, isOutput=False)
    wo_d = nc.declare_dram_parameter("wo", [32, 2 * D], bf, isOutput=False)
    cos_d = nc.declare_dram_parameter("rope_cos", [128, N], bf, isOutput=False)
    sin_d = nc.declare_dram_parameter("rope_sin", [128, N], bf, isOutput=False)
    bias_d = nc.declare_dram_parameter("biases", [128, 4], f32, isOutput=False)
    ident_d = nc.declare_dram_parameter("ident", [128, 128], bf, isOutput=False)
    out_d = nc.declare_dram_parameter("outT", [D, N], f32, isOutput=True)

    NT = N // 512  # 8 q-tiles of 512
    NCP = N // 256  # 16 chunk-pairs (2x128 k rows each)
    sched = _exp_schedule()

    with tile.TileContext(nc) as tc:
        from contextlib import ExitStack

        with ExitStack() as ctx:
            const = ctx.enter_context(tc.tile_pool(name="const", bufs=1))

            hs_sb = const.tile([128, 2, N], bf, tag="hs_sb")
            wqk_sb = const.tile([128, 2, 4, 64], bf, tag="wqk_sb")
            wv_sb = const.tile([128, 2, 64], bf, tag="wv_sb")
            wo_sb = const.tile([32, 2, D], bf, tag="wo_sb")
            cos_sb = const.tile([128, N], bf, tag="cos_sb")
            sin_sb = const.tile([128, N], bf, tag="sin_sb")
            bias_sb = const.tile([128, 4], f32, tag="bias_sb")
            schraud_sb = const.tile([128, 4], f32, tag="schraud_sb")
            ident_sb = const.tile([128, 128], bf, tag="ident_sb")
            qk_sb = const.tile([128, N], bf, tag="qk_sb")
            tmp_sb = const.tile([128, N], bf, tag="tmp_sb")
            Qd = const.tile([128, N], bf, tag="Qd")
            Kd = const.tile([128, N], bf, tag="Kd")
            # v tiles: [k-chunk 128, chunk, head, 35] with [v(32) | ones(1) | pad(2)]
            v_sb = const.tile([128, 32, 2, 35], bf, tag="v_sb")

            # --- input DMAs (critical-chain first; SP issues ~650ns each) ---
            for d in range(2):
                nc.sync.dma_start(
                    wqk_sb[:, d, :, :],
                    wqk_d[ts(d, 128), :].rearrange("p (w x) -> p w x", w=4))
            for d in range(2):
                nc.sync.dma_start(hs_sb[:, d, 0:512],
                                  hsT_d[ts(d, 128), 0:512])
            for d in range(2):
                nc.sync.dma_start(hs_sb[:, d, 512:1024],
                                  hsT_d[ts(d, 128), 512:1024])
            nc.sync.dma_start(cos_sb[:, 0:1024], cos_d[:, 0:1024])
            nc.sync.dma_start(sin_sb[:, 0:1024], sin_d[:, 0:1024])
            for d in range(2):
                nc.sync.dma_start(hs_sb[:, d, 1024:3072],
                                  hsT_d[ts(d, 128), 1024:3072])
            for d in range(2):
                nc.sync.dma_start(hs_sb[:, d, 3072:N],
                                  hsT_d[ts(d, 128), 3072:N])
            nc.sync.dma_start(cos_sb[:, 1024:N], cos_d[:, 1024:N])
            nc.sync.dma_start(sin_sb[:, 1024:N], sin_d[:, 1024:N])
            nc.sync.dma_start(bias_sb[:], bias_d[:])
            for d in range(2):
                nc.sync.dma_start(wv_sb[:, d, :], wv_d[ts(d, 128), :])
            nc.sync.dma_start(
                wo_sb[:], wo_d[:].rearrange("p (h j) -> p h j", h=2))
            nc.sync.dma_start(ident_sb[:], ident_d[:])
            nc.vector.tensor_scalar(
                schraud_sb[:], bias_sb[:], SCHRAUD_A, SCHRAUD_B,
                AluOpType.mult, AluOpType.add)
            nc.vector.memset(v_sb[:, :, :, 32:33], 1.0)

            # --- phase 3: attention main loop ---
            # step = (t, cp, h): one chunk-pair (2x128 k) x 512 q for one
            # head. Per step: 2 score matmuls (quadrant-packed) into the
            # two banks of one [128, 1024] psum tile, ONE 1024-wide exp op
            # (ACT exact or DVE Schraudolph; chunk pairs never straddle a
            # 512-block so the bias column is constant per tile), 8 PV
            # matmuls trailing pvd steps, and per-t front/tail work
            # deferred so PE never waits on the DVE chain.
            n_t = 0 if stage < 3 else (1 if stage == 3 else NT)
            with tc.tile_pool(name="ringp", bufs=3, space="PSUM") as ringp, \
                 tc.tile_pool(name="pvp", bufs=1, space="PSUM") as pvp, \
                 tc.tile_pool(name="tailp", bufs=1, space="PSUM") as tailp, \
                 tc.tile_pool(name="ptp", bufs=pvd + 5) as ptp, \
                 tc.tile_pool(name="osp", bufs=3) as osp, \
                 tc.tile_pool(name="rcp", bufs=3) as rcp, \
                 tc.tile_pool(name="ntp", bufs=8) as ntp:
                pv_tiles = {}

                # q/k dual projection (normal + half-swapped weight columns,
                # replacing rope's partition-swap DMAs) through the psum ring
                def emit_qkproj(t):
                    pr = ringp.tile([128, 1024], f32, tag="ring",
                                    name=f"qkproj{t}")
                    for d in range(2):
                        for w in range(4):
                            nc.tensor.matmul(
                                pr[64 * (w % 2):64 * (w % 2) + 64,
                                   512 * (w // 2):512 * (w // 2) + 512],
                                lhsT=wqk_sb[:, d, w, :],
                                rhs=hs_sb[:, d, ts(t, 512)],
                                start=(d == 0), stop=(d == 1),
                                tile_position=(0, 64 * (w % 2)))
                    nc.scalar.copy(qk_sb[:, ts(t, 512)], pr[:, 0:512])
                    nc.vector.tensor_copy(tmp_sb[:, ts(t, 512)],
                                          pr[:, 512:1024])

                # one 512-col rope slice (matches one projection tile)
                def emit_rope(sl, veng):
                    cs = ts(sl, 512)
                    veng.tensor_mul(tmp_sb[:, cs], tmp_sb[:, cs],
                                    sin_sb[:, cs])
                    veng.tensor_mul(qk_sb[:, cs], qk_sb[:, cs],
                                    cos_sb[:, cs])
                    veng.tensor_add(qk_sb[:, cs], qk_sb[:, cs],
                                    tmp_sb[:, cs])
                    # duplicate q and k to both halves (4-way row tiling)
                    nc.sync.dma_start(Qd[0:64, cs], qk_sb[0:64, cs])
                    nc.sync.dma_start(Qd[64:128, cs], qk_sb[0:64, cs])
                    nc.sync.dma_start(Kd[0:64, cs], qk_sb[64:128, cs])
                    nc.sync.dma_start(Kd[64:128, cs], qk_sb[64:128, cs])

                # v projection for chunks [lo, lo+n): n*64 f32 columns of a
                # ring tile; one start=True per psum bank, evicts batched 4
                def emit_vproj(vi, lo, n):
                    vt = ringp.tile([128, 16, 64], f32, tag="ring",
                                    name=f"vproj{vi}")
                    for j in range(n):
                        bf_ = (j % 8 == 0)
                        for d in range(2):
                            nc.tensor.matmul(
                                vt[:, j, :], lhsT=hs_sb[:, d, ts(lo + j, 128)],
                                rhs=wv_sb[:, d, :],
                                start=(d == 0 and bf_), stop=(d == 1),
                                skip_group_check=not bf_)
                    for g in range(0, n, 4):
                        src = vt[:, g:g + 4, :].rearrange(
                            "p a (h x) -> p a h x", h=2)
                        if (g // 4) % 2:
                            nc.scalar.copy(v_sb[:, lo + g:lo + g + 4, :, 0:32],
                                           src)
                        else:
                            nc.vector.tensor_copy(
                                v_sb[:, lo + g:lo + g + 4, :, 0:32], src)

                # pre-loop: enough projections/rope for the first ~8 steps;
                # slices 4,5 rope on the idle GpSimd engine
                for t_ in (0, 1, 2, 4, 5):
                    emit_qkproj(t_)
                for sl in (0, 1, 2):
                    emit_rope(sl, nc.vector)
                for sl in (4, 5):
                    emit_rope(sl, nc.gpsimd)
                emit_vproj(0, 0, 8)
                if stage < 4:
                    for t_ in (3, 6, 7):
                        emit_qkproj(t_)
                    for sl in (3, 6, 7):
                        emit_rope(sl, nc.vector)
                    emit_vproj(1, 8, 16)
                    emit_vproj(2, 24, 8)
                if stage == 1:
                    ob = out_d[:].bitcast(bf)
                    nc.sync.dma_start(ob[0:128, 0:N], qk_sb[:])
                    nc.sync.dma_start(
                        ob[128:256, 0:32 * 2 * 35],
                        v_sb[:].rearrange("p a b c -> p (a b c)"))
                if stage == 2:
                    ob = out_d[:].bitcast(bf)
                    nc.sync.dma_start(ob[0:128, 0:N], Qd[:])
                    nc.sync.dma_start(ob[128:256, 0:N], Kd[:])

                def emit_scores_exp(t, cp, h):
                    sp = ringp.tile([128, 1024], f32, tag="ring",
                                    name=f"sp{t}_{cp}_{h}")
                    same = (cp // 2) == t
                    for j in range(2):
                        c = 2 * cp + j
                        g = 2 * (c % 2) + h
                        nc.tensor.matmul(
                            sp[:, ts(j, 512)],
                            lhsT=Kd[ts(g, 32), ts(c, 128)],
                            rhs=Qd[ts(g, 32), ts(t, 512)],
                            start=True, stop=True,
                            tile_position=(32 * g, 0))
                    col = 2 * h + (0 if same else 1)
                    idx = (t * NCP + cp) * 2 + h
                    eng = sched[idx]
                    if eng == "A":
                        pt = ptp.tile([128, 1024], bf, tag="pt",
                                      name=f"pt{t}_{cp}_{h}")
                        nc.scalar.activation(
                            pt[:], sp[:], EXP,
                            bias=bias_sb[:, col:col + 1], scale=1.0)
                        return pt[:]
                    pt = ptp.tile([128, 1024], i16, tag="pt",
                                  name=f"pti{t}_{cp}_{h}")
                    nc.vector.tensor_scalar(
                        pt[:], sp[:], SCHRAUD_A,
                        schraud_sb[:, col:col + 1],
                        AluOpType.mult, AluOpType.add)
                    return pt[:].bitcast(bf)

                def emit_pv(t, cp, h, pts):
                    if cp == 0 and h == 0:
                        pv_tiles[t] = pvp.tile([128, 2, 4, 64], f32, tag="pv",
                                               name=f"pv{t}")
                    pv = pv_tiles[t]
                    for j in range(2):
                        c = 2 * cp + j
                        for qs in range(4):
                            first = (cp == 0 and h == 0 and j == 0
                                     and qs == 0)
                            last = (cp == NCP - 1 and j == 1)
                            nc.tensor.matmul(
                                pv[:, h, qs, 0:33],
                                lhsT=pts[:, 512 * j + 128 * qs:
                                         512 * j + 128 * qs + 128],
                                rhs=v_sb[:, c, h, 0:33],
                                start=first, stop=last,
                                skip_group_check=not first)

                def emit_front_recip(t):
                    pv = pv_tiles[t]
                    rc = rcp.tile([128, 2, 4], f32, tag="rc", name=f"rc{t}")
                    for h in range(2):
                        nc.vector.reciprocal_approx_fast(
                            out=rc[:, h, :], in_=pv[:, h, :, 32:33])
                    O = osp.tile([128, 2, 4, 32], bf, tag="O", name=f"O{t}")
                    return (t, pv, rc, O)

                def emit_front_norm(arg):
                    t, pv, rc, O, h = arg
                    for qs in range(4):
                        nc.vector.tensor_scalar(
                            O[:, h, qs, :], pv[:, h, qs, 0:32],
                            rc[:, h, qs:qs + 1], None, AluOpType.mult)

                def emit_tail_a(tail):
                    t, O = tail
                    tp = tailp.tile([32, 8, 128], bf, tag="tail",
                                    name=f"tp{t}")
                    numt = ntp.tile([32, 2, 512], bf, tag="numt",
                                    name=f"numt{t}")
                    for h in range(2):
                        for qs in range(4):
                            first = (h == 0 and qs == 0)
                            nc.tensor.matmul(
                                tp[:, 4 * h + qs, :], lhsT=O[:, h, qs, :],
                                rhs=ident_sb[:], is_transpose=True,
                                start=first, stop=True,
                                skip_group_check=not first)
                        nc.vector.tensor_copy(
                            numt[:, h, :],
                            tp[:, 4 * h:4 * h + 4, :].rearrange(
                                "p a b -> p (a b)"))
                    return (t, numt)

                def emit_tail_b(arg):
                    t, numt, jc = arg
                    op_ps = tailp.tile([128, 512], f32, tag="tail",
                                       name=f"op{t}_{jc}")
                    for h in range(2):
                        nc.tensor.matmul(
                            op_ps[:], lhsT=wo_sb[:, h, ts(jc, 128)],
                            rhs=numt[:, h, :],
                            start=(h == 0), stop=(h == 1))
                    op_sb = ntp.tile([128, 512], f32, tag="opsb",
                                     name=f"opsb{t}_{jc}")
                    nc.scalar.copy(op_sb[:], op_ps[:])
                    nc.sync.dma_start(
                        out_d[ts(jc, 128), ts(t, 512)], op_sb[:])

                from collections import deque
                pend_pv = deque()    # (t, c, pts), emitted pvd steps later
                todo = {}            # step_idx -> list of thunks
                step = 0

                def after(delay, fn, arg):
                    todo.setdefault(step + delay, []).append((fn, arg))

                def run_due():
                    for fn, arg in todo.pop(step, []):
                        fn(arg)

                def _tail_a(arg):
                    t, O, d0 = arg
                    t, numt = emit_tail_a((t, O))
                    after(d0, emit_tail_b, (t, numt, 0))
                    after(d0 + 1, emit_tail_b, (t, numt, 1))

                fast_state = {}

                def pop_pv():
                    t_, cp_, h_, pp_ = pend_pv.popleft()
                    emit_pv(t_, cp_, h_, pp_)
                    fast = (t_ == n_t - 1)
                    if cp_ == NCP - 1 and fast and n_t > 1:
                        # last q-tile: fine-grained per-head drain so the
                        # recip/norm/transpose/evict chains of the two heads
                        # overlap instead of serializing at the very end
                        pv = pv_tiles[t_]
                        if h_ == 0:
                            rc = rcp.tile([128, 2, 4], f32, tag="rc",
                                          name=f"rc{t_}")
                            O = osp.tile([128, 2, 4, 32], bf, tag="O",
                                         name=f"O{t_}")
                            tp = tailp.tile([32, 8, 128], bf, tag="tail",
                                            name=f"tp{t_}")
                            numt = ntp.tile([32, 2, 512], bf, tag="numt",
                                            name=f"numt{t_}")
                            fast_state.update(rc=rc, O=O, tp=tp, numt=numt)
                        else:
                            rc, O = fast_state["rc"], fast_state["O"]
                            tp, numt = fast_state["tp"], fast_state["numt"]
                        nc.vector.reciprocal_approx_fast(
                            out=rc[:, h_, :], in_=pv[:, h_, :, 32:33])
                        for qs in range(4):
                            nc.vector.tensor_scalar(
                                O[:, h_, qs, :], pv[:, h_, qs, 0:32],
                                rc[:, h_, qs:qs + 1], None, AluOpType.mult)
                        for qs in range(4):
                            first = (h_ == 0 and qs == 0)
                            nc.tensor.matmul(
                                tp[:, 4 * h_ + qs, :], lhsT=O[:, h_, qs, :],
                                rhs=ident_sb[:], is_transpose=True,
                                start=first, stop=True,
                                skip_group_check=not first)
                        nc.vector.tensor_copy(
                            numt[:, h_, :],
                            tp[:, 4 * h_:4 * h_ + 4, :].rearrange(
                                "p a b -> p (a b)"))
                        if h_ == 1:
                            emit_tail_b((t_, numt, 0))
                            emit_tail_b((t_, numt, 1))
                        return
                    if cp_ == NCP - 1 and h_ == 1:
                        # recip+norm immediately (DVE was kept free of exp
                        # work for this chunk-pair) so the single pv bank
                        # frees before pv(t+1) needs it; PE-side tail ops
                        # follow a few steps later off the scores ring
                        t2, pv2, rc2, O2 = emit_front_recip(t_)
                        emit_front_norm((t2, pv2, rc2, O2, 0))
                        emit_front_norm((t2, pv2, rc2, O2, 1))
                        if t_ == n_t - 1:
                            after(1, _tail_a, (t2, O2, 1))
                        else:
                            after(5, _tail_a, (t2, O2, 2))

                for t in range(n_t):
                    pvd_eff = 1 if t == n_t - 1 else pvd
                    for cp in range(NCP):
                        for h in range(2):
                            pend_pv.append(
                                (t, cp, h, emit_scores_exp(t, cp, h)))
                            # give the head PV 2 extra steps when it opens a
                            # new q-tile (waits on the pv bank being freed by
                            # the previous tile's norm chain)
                            lim = pvd_eff
                            if pend_pv and pend_pv[0][1] == 0 \
                                    and pend_pv[0][0] > 0:
                                lim = pvd_eff + 4
                            while len(pend_pv) > lim:
                                pop_pv()
                            run_due()
                            step += 1
                            if stage >= 4 and t == 0 and h == 1:
                                if cp == 0:
                                    emit_qkproj(3)
                                elif cp == 1:
                                    emit_rope(3, nc.vector)
                                    emit_vproj(1, 8, 16)
                                elif cp == 2:
                                    emit_qkproj(6)
                                elif cp == 4:
                                    emit_rope(6, nc.vector)
                                elif cp == 5:
                                    emit_qkproj(7)
                                    emit_vproj(2, 24, 8)
                                elif cp == 7:
                                    emit_rope(7, nc.vector)
                while pend_pv:
                    pop_pv()
                while todo:
                    run_due()
                    step += 1
    nc.compile()
    return nc


def _rope_tables():
    j = np.arange(8, dtype=np.float64)
    inv = 10000.0 ** (-(2.0 * j / HD))  # [8]
    ang = np.arange(N, dtype=np.float64)[None, :] * inv[:, None]  # [8, N]
    cosb = np.ones((32, N), dtype=np.float64)
    sinb = np.zeros((32, N), dtype=np.float64)
    cosb[0:8] = np.cos(ang)
    cosb[16:24] = np.cos(ang)
    sinb[0:8] = -np.sin(ang)
    sinb[16:24] = np.sin(ang)
    cos128 = np.tile(cosb, (4, 1)).astype(BF16)
    sin128 = np.tile(sinb, (4, 1)).astype(BF16)
    return cos128, sin128


def kernel(**inputs):
    hs = np.asarray(inputs["hidden_states"], dtype=np.float32)
    qw = np.asarray(inputs["q_w"], dtype=np.float32)
    kw = np.asarray(inputs["k_w"], dtype=np.float32)
    vw = np.asarray(inputs["v_w"], dtype=np.float32)
    ow = np.asarray(inputs["o_w"], dtype=np.float32)
    ob = np.asarray(inputs["o_b"], dtype=np.float32)
    qb = np.asarray(inputs["q_b"], dtype=np.float32)
    kb = np.asarray(inputs["k_b"], dtype=np.float32)
    vb = np.asarray(inputs["v_b"], dtype=np.float32)
    ab = np.asarray(inputs["attention_biases"], dtype=np.float32)
    seq = int(np.asarray(inputs["sequence_length"]))
    assert seq == SEQ, f"kernel compiled for sequence_length={SEQ}, got {seq}"
    assert hs.shape == (B, N, D)
    assert not (np.any(qb) or np.any(kb) or np.any(vb)), "nonzero qkv bias unsupported"

    stage = int(os.environ.get("KERNEL_STAGE", "4"))
    if ("nc", stage) not in _NC_CACHE:
        _NC_CACHE[("nc", stage)] = _build_nc(stage)
    nc = _NC_CACHE[("nc", stage)]

    cos128, sin128 = _rope_tables()
    ident = np.eye(128, dtype=np.float32).astype(BF16)
    # half-swap of each head's 32 output dims (partner rows for rope)
    perm = np.concatenate([np.arange(16, 32), np.arange(16)])
    perm64 = np.concatenate([perm, perm + 32])
    in_maps = []
    for c in range(NCORES):
        b = c // 4
        h0 = 2 * (c % 4)
        rows = slice(h0 * HD, h0 * HD + 2 * HD)
        qwr = qw[rows, :] * SCALE
        kwr = kw[rows, :]
        wqk = np.concatenate(
            [qwr.T, kwr.T, qwr[perm64].T, kwr[perm64].T], axis=1)
        in_maps.append({
            "hsT": np.ascontiguousarray(hs[b].T).astype(BF16),
            "wqk": np.ascontiguousarray(wqk).astype(BF16),
            "wv": np.ascontiguousarray(vw[rows, :].T).astype(BF16),
            "wo": np.ascontiguousarray(
                ow[:, rows].T.reshape(2, 32, D).transpose(1, 0, 2)
                .reshape(32, 2 * D)).astype(BF16),
            "rope_cos": cos128,
            "rope_sin": sin128,
            "ident": ident,
            "biases": np.ascontiguousarray(
                np.broadcast_to(ab[h0:h0 + 2].reshape(1, 4), (128, 4))
            ).astype(np.float32),
        })

    global _LAST_IN_MAPS, _LAST_RESULTS
    _LAST_IN_MAPS = in_maps
    from concourse.bass_utils import run_bass_kernel_spmd
    res = run_bass_kernel_spmd(nc, in_maps, core_ids=list(range(NCORES)))
    _LAST_RESULTS = res.results
    out = np.zeros((B, N, D), dtype=np.float32)
    for c in range(NCORES):
        out[c // 4] += res.results[c]["outT"].T.astype(np.float32)
    out += ob[None, None, :]
    return out
